# revision 7
# baseline (speedup 1.0000x reference)
"""LocationAwareAttention Trainium2 kernel (8 NeuronCores, batch-data-parallel).

Reference computation (per batch b):
    conv[t, h] = sum_k conv_w[h, 0, k] * la[t + k - 1]          (padded conv1d)
    hidden[t, d] = tanh(q[d] + (value @ Wv^T)[t, d] + conv[t, d] + conv_b[d] + bias[d])
    score[t] = sigmoid(hidden[t, :] @ score_w + score_b)
    attn = score / sum(score);  context = attn @ value

Device dataflow (per core, 4 batches):
    hiddenT[d, t] = tanh( sum_h WvT[h, d] * valueT[h, t]        (PE, K=512 in 4 chunks)
                        + sum_k s3[k, d] * r[k, t]              (PE, K=3 conv fold)
                        + qb[d] )                               (ACT per-partition bias)
    score = M=1 PE matmuls over hiddenT; sigmoid+row-sum fused on ACT (accum_out)
    attn normalize on DVE; attn^T via a DRAM bounce; context = M=1 PE matmuls
    over naturally-laid-out value tiles.
value is shipped twice from host (transposed for the projection, natural for the
context) so no on-device transposes are needed.
"""

import os
import sys

try:
    import concourse  # noqa: F401
except ImportError:
    for _p in ("/opt/trn_rl_repo", "/root/.axon_site/_ro/trn_rl_repo"):
        if os.path.isdir(_p):
            sys.path.insert(0, _p)
            break

import numpy as np

import concourse.bacc as bacc
import concourse.mybir as mybir
import concourse.tile as tile
from concourse.bass_utils import run_bass_kernel_spmd

F32 = mybir.dt.float32
F32R = mybir.dt.float32r  # fp32 bytes; single-pass PE mode (4x faster than fp32 LOW_HIGH)
AF = mybir.ActivationFunctionType

N_CORES = 8
B, VL, H = 32, 2048, 512
BPC = B // N_CORES  # batches per core
KC = H // 128  # contraction chunks
TJ = VL // 512  # 512-wide t chunks

_CACHE = {}


def build_module():
    nc = bacc.Bacc("TRN2", target_bir_lowering=False, debug=False, num_devices=N_CORES)

    valt_d = nc.dram_tensor("valt", [BPC, H, VL], F32R, kind="ExternalInput")
    vnat_d = nc.dram_tensor("vnat", [BPC, VL, H], F32R, kind="ExternalInput")
    wvt_d = nc.dram_tensor("wvt", [128, KC, H], F32R, kind="ExternalInput")
    wqt_d = nc.dram_tensor("wqt", [128, KC, H], F32R, kind="ExternalInput")
    qt_d = nc.dram_tensor("qt", [128, KC, BPC], F32R, kind="ExternalInput")
    swt_d = nc.dram_tensor("swt", [128, KC], F32R, kind="ExternalInput")
    bc4_d = nc.dram_tensor("bc4", [128, KC], F32, kind="ExternalInput")
    s3_d = nc.dram_tensor("s3", [3, H], F32R, kind="ExternalInput")
    rmat_d = nc.dram_tensor("rmat", [BPC, 3, VL], F32R, kind="ExternalInput")
    sb_d = nc.dram_tensor("sb", [1, 1], F32, kind="ExternalInput")

    ctx_d = nc.dram_tensor("ctx", [BPC, H], F32, kind="ExternalOutput")
    attn_d = nc.dram_tensor("attn", [BPC, VL], F32, kind="ExternalOutput")

    vnr = vnat_d.ap().rearrange("b (tj j2 p) h -> b tj p j2 h", j2=4, p=128)

    with tile.TileContext(nc) as tc:
        with (
            tc.tile_pool(name="weights", bufs=1) as wpool,
            tc.tile_pool(name="vt", bufs=2 * KC) as vt_pool,
            tc.tile_pool(name="vn", bufs=2 * TJ) as vn_pool,
            tc.tile_pool(name="rb", bufs=2) as r_pool,
            tc.tile_pool(name="hid", bufs=3) as h_pool,
            tc.tile_pool(name="small", bufs=2) as sc_pool,
            tc.tile_pool(name="attnT", bufs=2) as at_pool,
            tc.tile_pool(name="dbounce", bufs=2, space="DRAM") as dram_pool,
            tc.tile_pool(name="pre_ps", bufs=2, space="PSUM") as ppool,
            tc.tile_pool(name="score_ps", bufs=2, space="PSUM") as spsum_pool,
            tc.tile_pool(name="ctx_ps", bufs=2, space="PSUM") as cpsum_pool,
            tc.tile_pool(name="q_ps", bufs=1, space="PSUM") as qpsum_pool,
        ):
            # --- load weights / small constants (once per core) ---
            wvt_sb = wpool.tile([128, KC, H], F32R)
            nc.sync.dma_start(wvt_sb[:], wvt_d[:])
            wqt_sb = wpool.tile([128, KC, H], F32R)
            nc.sync.dma_start(wqt_sb[:], wqt_d[:])
            qt_sb = wpool.tile([128, KC, BPC], F32R)
            nc.sync.dma_start(qt_sb[:], qt_d[:])
            swt_sb = wpool.tile([128, KC], F32R)
            nc.sync.dma_start(swt_sb[:], swt_d[:])
            bc4_sb = wpool.tile([128, KC], F32)
            nc.sync.dma_start(bc4_sb[:], bc4_d[:])
            s3_sb = wpool.tile([3, H], F32R)
            nc.sync.dma_start(s3_sb[:], s3_d[:])
            sb_sb = wpool.tile([1, 1], F32)
            nc.sync.dma_start(sb_sb[:], sb_d[:])

            # --- q projection for all batches: qT[d, b] = sum_h Wq[d,h] query[b,h]
            q_ps = qpsum_pool.tile([128, KC * BPC], F32)
            for dc in range(KC):
                for kc in range(KC):
                    nc.tensor.matmul(
                        q_ps[:, dc * BPC : (dc + 1) * BPC],
                        wqt_sb[:, kc, dc * 128 : (dc + 1) * 128],
                        qt_sb[:, kc, :],
                        start=(kc == 0),
                        stop=(kc == KC - 1),
                    )
            # qb[d, b] = qT[d, b] + bias[d] + conv_b[d]   (tanh bias, per-partition)
            qb_sb = wpool.tile([128, KC, BPC], F32)
            for dc in range(KC):
                nc.vector.tensor_tensor(
                    qb_sb[:, dc, :],
                    q_ps[:, dc * BPC : (dc + 1) * BPC],
                    bc4_sb[:, dc : dc + 1].to_broadcast((128, BPC)),
                    mybir.AluOpType.add,
                )

            def emit_loads(b, split_vt=False):
                # vt on Sync HWDGE, vn on GpSimd SWDGE: independent queues so a
                # blocked issue on one never head-of-line-blocks the other.
                vts, vns = [], []
                for hc in range(KC):
                    vt = vt_pool.tile([128, VL], F32R, tag="vt")
                    if split_vt:  # first batch: quarter-DMAs so PE starts sooner
                        for j in range(TJ):
                            nc.sync.dma_start(
                                vt[:, j * 512 : (j + 1) * 512],
                                valt_d[b, hc * 128 : (hc + 1) * 128,
                                       j * 512 : (j + 1) * 512],
                            )
                    else:
                        nc.sync.dma_start(
                            vt[:], valt_d[b, hc * 128 : (hc + 1) * 128, :]
                        )
                    vts.append(vt)
                for tj in range(TJ):
                    vn = vn_pool.tile([128, 4, 512], F32R, tag="vn")
                    nc.gpsimd.dma_start(vn[:], vnr[b, tj])
                    vns.append(vn)
                r_b = r_pool.tile([3, VL], F32R, tag="r_b")
                nc.sync.dma_start(r_b[:], rmat_d[b])
                return vts, vns, r_b

            loads = {0: emit_loads(0, split_vt=True)}
            prev = None
            for b in range(BPC):
                vts, vns, r_b = loads.pop(b)
                if b + 1 < BPC:
                    # prefetch next batch now, so its DMAs sit AHEAD of this
                    # batch's bounce hops in both queues
                    loads[b + 1] = emit_loads(b + 1)

                score_sb = sc_pool.tile([1, VL], F32R, tag="score_sb")
                ssum4 = sc_pool.tile([1, TJ], F32, tag="ssum4")
                ab = dram_pool.tile([1, VL], F32R)
                attnT = at_pool.tile([128, VL // 128], F32R)
                sps = {}
                pending = None

                def flush_score(j, dc, hT):
                    nc.tensor.matmul(
                        sps[j][:],
                        swt_sb[:, dc : dc + 1],
                        hT[:],
                        start=(dc == 0),
                        stop=(dc == KC - 1),
                    )
                    if dc == KC - 1:
                        # sigmoid(score + score_b) with fused row-sum; then
                        # bounce raw scores toward attn^T layout. hop1 is on
                        # ACT right behind the sigmoid (never waits); hop2's
                        # 4B-gather runs on the idle GpSimd queue.
                        nc.scalar.activation(
                            score_sb[0:1, j * 512 : (j + 1) * 512],
                            sps[j][:],
                            AF.Sigmoid,
                            bias=sb_sb[0:1, :],
                            accum_out=ssum4[0:1, j : j + 1],
                        )
                        nc.scalar.dma_start(
                            ab[0:1, j * 512 : (j + 1) * 512],
                            score_sb[0:1, j * 512 : (j + 1) * 512],
                        )
                        nc.gpsimd.dma_start(
                            attnT[:, j * 4 : (j + 1) * 4],
                            ab[0:1, j * 512 : (j + 1) * 512].rearrange(
                                "1 (j2 p) -> p j2", p=128
                            ),
                        )

                for j in range(TJ):
                    sps[j] = spsum_pool.tile([1, 512], F32, tag="sp", name=f"sp_{b}_{j}")
                    for dc in range(KC):
                        pre = ppool.tile([128, 512], F32)
                        for kc in range(KC):
                            nc.tensor.matmul(
                                pre[:],
                                wvt_sb[:, kc, dc * 128 : (dc + 1) * 128],
                                vts[kc][:, j * 512 : (j + 1) * 512],
                                start=(kc == 0),
                                stop=False,
                            )
                        nc.tensor.matmul(
                            pre[:],
                            s3_sb[:, dc * 128 : (dc + 1) * 128],
                            r_b[:, j * 512 : (j + 1) * 512],
                            start=False,
                            stop=True,
                        )
                        hT = h_pool.tile([128, 512], F32R)
                        nc.scalar.activation(
                            hT[:], pre[:], AF.Tanh, bias=qb_sb[:, dc, b : b + 1]
                        )
                        # score matmul delayed one (j, dc) block so PE never
                        # waits on the tanh that feeds it
                        if pending is not None:
                            flush_score(*pending)
                        pending = (j, dc, hT)
                        # previous batch's context right after the first block,
                        # freeing its vn tiles for the next prefetch
                        if j == 0 and dc == 0 and prev is not None:
                            _emit_ctx(nc, cpsum_pool, sc_pool, ctx_d, *prev)
                flush_score(*pending)

                # --- normalized attn output (off the ctx critical path) ---
                ssum = sc_pool.tile([1, 1], F32, tag="ssum")
                nc.vector.tensor_reduce(
                    ssum[:], ssum4[:], axis=mybir.AxisListType.X, op=mybir.AluOpType.add
                )
                recip = sc_pool.tile([1, 1], F32, tag="recip")
                nc.vector.reciprocal(recip[:], ssum[:])
                attn_sb = sc_pool.tile([1, VL], F32, tag="attn_sb")
                nc.vector.tensor_scalar_mul(attn_sb[:], score_sb[:], recip[0:1, :])
                nc.sync.dma_start(attn_d[b : b + 1, :], attn_sb[:])

                prev = (b, attnT, vns, recip)

            _emit_ctx(nc, cpsum_pool, sc_pool, ctx_d, *prev)

    nc.compile()
    return nc


def _emit_ctx(nc, cpsum_pool, sc_pool, ctx_d, b, attnT, vns, recip):
    # ctx_u = sum_t sigmoid_score[t] * value[t, :]; normalized by 1/sum at copy
    cp = cpsum_pool.tile([1, 512], F32)
    n = VL // 128
    for jj in range(n):
        nc.tensor.matmul(
            cp[:],
            attnT[:, jj : jj + 1],
            vns[jj // 4][:, jj % 4, :],
            start=(jj == 0),
            stop=(jj == n - 1),
        )
    ctx_sb = sc_pool.tile([1, 512], F32, tag="ctx_sb")
    nc.scalar.activation(ctx_sb[:], cp[:], AF.Copy, scale=recip[0:1, :])
    nc.sync.dma_start(ctx_d[b : b + 1, :], ctx_sb[:])


def make_in_maps(query, value, last_attn, conv_w, conv_b, Wq, Wv, score_w, score_b, bias):
    query = np.asarray(query, dtype=np.float32)
    value = np.ascontiguousarray(np.asarray(value, dtype=np.float32))
    last_attn = np.asarray(last_attn, dtype=np.float32)
    conv_w = np.asarray(conv_w, dtype=np.float32)
    conv_b = np.asarray(conv_b, dtype=np.float32)
    Wq = np.asarray(Wq, dtype=np.float32)
    Wv = np.asarray(Wv, dtype=np.float32)
    score_w = np.asarray(score_w, dtype=np.float32)
    score_b = np.asarray(score_b, dtype=np.float32)
    bias = np.asarray(bias, dtype=np.float32)

    # Shared (replicated) weight-side arrays.
    # wvt[p, kc, d] = Wv[d, kc*128+p]  (lhsT chunks for the projection)
    wvt = np.ascontiguousarray(Wv.T.reshape(KC, 128, H).transpose(1, 0, 2))
    wqt = np.ascontiguousarray(Wq.T.reshape(KC, 128, H).transpose(1, 0, 2))
    swt = np.ascontiguousarray(score_w[0].reshape(KC, 128).T)
    bc4 = np.ascontiguousarray((bias + conv_b).reshape(KC, 128).T)
    s3 = np.ascontiguousarray(conv_w[:, 0, :].T)  # s3[k, d] = conv_w[d, 0, k]
    sb = score_b.reshape(1, 1)

    # r[b, k, t] = la[b, t + k - 1] (zero-padded)
    r = np.zeros((B, 3, VL), dtype=np.float32)
    r[:, 0, 1:] = last_attn[:, :-1]
    r[:, 1, :] = last_attn
    r[:, 2, :-1] = last_attn[:, 1:]

    qT = query.reshape(B, H).T  # [h, b]

    in_maps = []
    for c in range(N_CORES):
        s = slice(c * BPC, (c + 1) * BPC)
        vshard = value[s]
        in_maps.append(
            {
                "valt": np.ascontiguousarray(vshard.transpose(0, 2, 1)),
                "vnat": vshard,
                "wvt": wvt,
                "wqt": wqt,
                "qt": np.ascontiguousarray(
                    qT[:, s].reshape(KC, 128, BPC).transpose(1, 0, 2)
                ),
                "swt": swt,
                "bc4": bc4,
                "s3": s3,
                "rmat": r[s],
                "sb": sb,
            }
        )
    return in_maps


def kernel(query, value, last_attn, conv_w, conv_b, Wq, Wv, score_w, score_b, bias,
           _trace=False):
    if "nc" not in _CACHE:
        _CACHE["nc"] = build_module()
    nc = _CACHE["nc"]

    in_maps = make_in_maps(
        query, value, last_attn, conv_w, conv_b, Wq, Wv, score_w, score_b, bias
    )
    res = run_bass_kernel_spmd(nc, in_maps, list(range(N_CORES)), trace=_trace)

    context = np.concatenate([res.results[c]["ctx"] for c in range(N_CORES)], axis=0)
    attn = np.concatenate([res.results[c]["attn"] for c in range(N_CORES)], axis=0)
    out = (context.reshape(B, 1, H).astype(np.float32), attn.astype(np.float32))
    if _trace:
        return out, res
    return out


# revision 8
# speedup vs baseline: 1.2503x; 1.2503x over previous
"""LocationAwareAttention Trainium2 kernel (8 NeuronCores, batch-data-parallel).

Reference computation (per batch b):
    conv[t, h] = sum_k conv_w[h, 0, k] * la[t + k - 1]          (padded conv1d)
    hidden[t, d] = tanh(q[d] + (value @ Wv^T)[t, d] + conv[t, d] + conv_b[d] + bias[d])
    score[t] = sigmoid(hidden[t, :] @ score_w + score_b)
    attn = score / sum(score);  context = attn @ value

Device dataflow (per core, 4 batches):
    hiddenT[d, t] = tanh( sum_h WvT[h, d] * valueT[h, t]        (PE, K=512 in 4 chunks)
                        + sum_k s3[k, d] * r[k, t]              (PE, K=3 conv fold)
                        + qb[d] )                               (ACT per-partition bias)
    score = M=1 PE matmuls over hiddenT; sigmoid+row-sum fused on ACT (accum_out)
    attn normalize on DVE; attn^T via a DRAM bounce; context = M=1 PE matmuls
    over naturally-laid-out value tiles.
value is shipped twice from host (transposed for the projection, natural for the
context) so no on-device transposes are needed.
"""

import os
import sys

try:
    import concourse  # noqa: F401
except ImportError:
    for _p in ("/opt/trn_rl_repo", "/root/.axon_site/_ro/trn_rl_repo"):
        if os.path.isdir(_p):
            sys.path.insert(0, _p)
            break

import numpy as np

import concourse.bacc as bacc
import concourse.mybir as mybir
import concourse.tile as tile
from concourse.bass_utils import run_bass_kernel_spmd

F32 = mybir.dt.float32
F32R = mybir.dt.float32r  # fp32 bytes; single-pass PE mode (4x faster than fp32 LOW_HIGH)
AF = mybir.ActivationFunctionType

N_CORES = 8
B, VL, H = 32, 2048, 512
BPC = B // N_CORES  # batches per core
KC = H // 128  # contraction chunks
TJ = VL // 512  # 512-wide t chunks

_CACHE = {}


def build_module():
    nc = bacc.Bacc("TRN2", target_bir_lowering=False, debug=False, num_devices=N_CORES)

    valt_d = nc.dram_tensor("valt", [BPC, H, VL], F32R, kind="ExternalInput")
    vnat_d = nc.dram_tensor("vnat", [BPC, VL, H], F32R, kind="ExternalInput")
    wvt_d = nc.dram_tensor("wvt", [128, KC, H], F32R, kind="ExternalInput")
    wqt_d = nc.dram_tensor("wqt", [128, KC, H], F32R, kind="ExternalInput")
    qt_d = nc.dram_tensor("qt", [128, KC, BPC], F32R, kind="ExternalInput")
    swt_d = nc.dram_tensor("swt", [128, KC], F32R, kind="ExternalInput")
    bc4_d = nc.dram_tensor("bc4", [128, KC], F32, kind="ExternalInput")
    s3_d = nc.dram_tensor("s3", [3, H], F32R, kind="ExternalInput")
    rmat_d = nc.dram_tensor("rmat", [BPC, 3, VL], F32R, kind="ExternalInput")
    sb_d = nc.dram_tensor("sb", [1, 1], F32, kind="ExternalInput")

    ctx_d = nc.dram_tensor("ctx", [BPC, H], F32, kind="ExternalOutput")
    attn_d = nc.dram_tensor("attn", [BPC, VL], F32, kind="ExternalOutput")

    vnr = vnat_d.ap().rearrange("b (tj j2 p) h -> b tj p j2 h", j2=4, p=128)

    with tile.TileContext(nc) as tc:
        with (
            tc.tile_pool(name="weights", bufs=1) as wpool,
            tc.tile_pool(name="vt", bufs=2 * KC) as vt_pool,
            tc.tile_pool(name="vn", bufs=2 * TJ) as vn_pool,
            tc.tile_pool(name="rb", bufs=2) as r_pool,
            tc.tile_pool(name="hid", bufs=3) as h_pool,
            tc.tile_pool(name="small", bufs=2) as sc_pool,
            tc.tile_pool(name="attnT", bufs=2) as at_pool,
            tc.tile_pool(name="dbounce", bufs=2, space="DRAM") as dram_pool,
            tc.tile_pool(name="pre_ps", bufs=2, space="PSUM") as ppool,
            tc.tile_pool(name="score_ps", bufs=2, space="PSUM") as spsum_pool,
            tc.tile_pool(name="ctx_ps", bufs=2, space="PSUM") as cpsum_pool,
            tc.tile_pool(name="q_ps", bufs=1, space="PSUM") as qpsum_pool,
        ):
            # --- load weights / small constants (once per core) ---
            wvt_sb = wpool.tile([128, KC, H], F32R)
            nc.sync.dma_start(wvt_sb[:], wvt_d[:])
            wqt_sb = wpool.tile([128, KC, H], F32R)
            nc.sync.dma_start(wqt_sb[:], wqt_d[:])
            qt_sb = wpool.tile([128, KC, BPC], F32R)
            nc.sync.dma_start(qt_sb[:], qt_d[:])
            swt_sb = wpool.tile([128, KC], F32R)
            nc.sync.dma_start(swt_sb[:], swt_d[:])
            bc4_sb = wpool.tile([128, KC], F32)
            nc.sync.dma_start(bc4_sb[:], bc4_d[:])
            s3_sb = wpool.tile([3, H], F32R)
            nc.sync.dma_start(s3_sb[:], s3_d[:])
            sb_sb = wpool.tile([1, 1], F32)
            nc.sync.dma_start(sb_sb[:], sb_d[:])

            # --- q projection for all batches: qT[d, b] = sum_h Wq[d,h] query[b,h]
            q_ps = qpsum_pool.tile([128, KC * BPC], F32)
            for dc in range(KC):
                for kc in range(KC):
                    nc.tensor.matmul(
                        q_ps[:, dc * BPC : (dc + 1) * BPC],
                        wqt_sb[:, kc, dc * 128 : (dc + 1) * 128],
                        qt_sb[:, kc, :],
                        start=(kc == 0),
                        stop=(kc == KC - 1),
                    )
            # qb[d, b] = qT[d, b] + bias[d] + conv_b[d]   (tanh bias, per-partition)
            qb_sb = wpool.tile([128, KC, BPC], F32)
            for dc in range(KC):
                nc.vector.tensor_tensor(
                    qb_sb[:, dc, :],
                    q_ps[:, dc * BPC : (dc + 1) * BPC],
                    bc4_sb[:, dc : dc + 1].to_broadcast((128, BPC)),
                    mybir.AluOpType.add,
                )

            def emit_loads(b, split_vt=False):
                # vt on Sync HWDGE, vn on GpSimd SWDGE: independent queues so a
                # blocked issue on one never head-of-line-blocks the other.
                vts, vns = [], []
                for hc in range(KC):
                    vt = vt_pool.tile([128, VL], F32R, tag="vt")
                    if split_vt:  # first batch: quarter-DMAs so PE starts sooner
                        for j in range(TJ):
                            nc.sync.dma_start(
                                vt[:, j * 512 : (j + 1) * 512],
                                valt_d[b, hc * 128 : (hc + 1) * 128,
                                       j * 512 : (j + 1) * 512],
                            )
                    else:
                        nc.sync.dma_start(
                            vt[:], valt_d[b, hc * 128 : (hc + 1) * 128, :]
                        )
                    vts.append(vt)
                for tj in range(TJ):
                    vn = vn_pool.tile([128, 4, 512], F32R, tag="vn")
                    nc.gpsimd.dma_start(vn[:], vnr[b, tj])
                    vns.append(vn)
                r_b = r_pool.tile([3, VL], F32R, tag="r_b")
                nc.sync.dma_start(r_b[:], rmat_d[b])
                return vts, vns, r_b

            loads = {0: emit_loads(0)}
            prev = None
            for b in range(BPC):
                vts, vns, r_b = loads.pop(b)
                if b + 1 < BPC:
                    # prefetch next batch now, so its DMAs sit AHEAD of this
                    # batch's bounce hops in both queues
                    loads[b + 1] = emit_loads(b + 1)

                score_sb = sc_pool.tile([1, VL], F32R, tag="score_sb")
                ssum4 = sc_pool.tile([1, TJ], F32, tag="ssum4")
                ab = dram_pool.tile([1, VL], F32R)
                attnT = at_pool.tile([128, VL // 128], F32R)
                sps = {}
                pending = None

                def flush_score(j, dc, hT):
                    nc.tensor.matmul(
                        sps[j][:],
                        swt_sb[:, dc : dc + 1],
                        hT[:],
                        start=(dc == 0),
                        stop=(dc == KC - 1),
                    )
                    if dc == KC - 1:
                        # sigmoid(score + score_b) with fused row-sum; then
                        # bounce raw scores toward attn^T layout. hop1 is on
                        # ACT right behind the sigmoid (never waits); hop2's
                        # 4B-gather runs on the idle GpSimd queue.
                        nc.scalar.activation(
                            score_sb[0:1, j * 512 : (j + 1) * 512],
                            sps[j][:],
                            AF.Sigmoid,
                            bias=sb_sb[0:1, :],
                            accum_out=ssum4[0:1, j : j + 1],
                        )
                        nc.scalar.dma_start(
                            ab[0:1, j * 512 : (j + 1) * 512],
                            score_sb[0:1, j * 512 : (j + 1) * 512],
                        )
                        nc.gpsimd.dma_start(
                            attnT[:, j * 4 : (j + 1) * 4],
                            ab[0:1, j * 512 : (j + 1) * 512].rearrange(
                                "1 (j2 p) -> p j2", p=128
                            ),
                        )

                for j in range(TJ):
                    sps[j] = spsum_pool.tile([1, 512], F32, tag="sp", name=f"sp_{b}_{j}")
                    for dc in range(KC):
                        pre = ppool.tile([128, 512], F32)
                        for kc in range(KC):
                            nc.tensor.matmul(
                                pre[:],
                                wvt_sb[:, kc, dc * 128 : (dc + 1) * 128],
                                vts[kc][:, j * 512 : (j + 1) * 512],
                                start=(kc == 0),
                                stop=False,
                            )
                        nc.tensor.matmul(
                            pre[:],
                            s3_sb[:, dc * 128 : (dc + 1) * 128],
                            r_b[:, j * 512 : (j + 1) * 512],
                            start=False,
                            stop=True,
                        )
                        hT = h_pool.tile([128, 512], F32R)
                        nc.scalar.activation(
                            hT[:], pre[:], AF.Tanh, bias=qb_sb[:, dc, b : b + 1]
                        )
                        # score matmul delayed one (j, dc) block so PE never
                        # waits on the tanh that feeds it
                        if pending is not None:
                            flush_score(*pending)
                        pending = (j, dc, hT)
                        # previous batch's context right after the first block,
                        # freeing its vn tiles for the next prefetch
                        if j == 0 and dc == KC - 1 and prev is not None:
                            _emit_ctx(nc, cpsum_pool, sc_pool, ctx_d, *prev)
                flush_score(*pending)

                # --- normalized attn output (off the ctx critical path) ---
                ssum = sc_pool.tile([1, 1], F32, tag="ssum")
                nc.vector.tensor_reduce(
                    ssum[:], ssum4[:], axis=mybir.AxisListType.X, op=mybir.AluOpType.add
                )
                recip = sc_pool.tile([1, 1], F32, tag="recip")
                nc.vector.reciprocal(recip[:], ssum[:])
                attn_sb = sc_pool.tile([1, VL], F32, tag="attn_sb")
                nc.vector.tensor_scalar_mul(attn_sb[:], score_sb[:], recip[0:1, :])
                nc.sync.dma_start(attn_d[b : b + 1, :], attn_sb[:])

                prev = (b, attnT, vns, recip)

            _emit_ctx(nc, cpsum_pool, sc_pool, ctx_d, *prev)

    nc.compile()
    return nc


def _emit_ctx(nc, cpsum_pool, sc_pool, ctx_d, b, attnT, vns, recip):
    # ctx_u = sum_t sigmoid_score[t] * value[t, :]; normalized by 1/sum at copy
    cp = cpsum_pool.tile([1, 512], F32)
    n = VL // 128
    for jj in range(n):
        nc.tensor.matmul(
            cp[:],
            attnT[:, jj : jj + 1],
            vns[jj // 4][:, jj % 4, :],
            start=(jj == 0),
            stop=(jj == n - 1),
        )
    ctx_sb = sc_pool.tile([1, 512], F32, tag="ctx_sb")
    nc.scalar.activation(ctx_sb[:], cp[:], AF.Copy, scale=recip[0:1, :])
    nc.sync.dma_start(ctx_d[b : b + 1, :], ctx_sb[:])


def make_in_maps(query, value, last_attn, conv_w, conv_b, Wq, Wv, score_w, score_b, bias):
    query = np.asarray(query, dtype=np.float32)
    value = np.ascontiguousarray(np.asarray(value, dtype=np.float32))
    last_attn = np.asarray(last_attn, dtype=np.float32)
    conv_w = np.asarray(conv_w, dtype=np.float32)
    conv_b = np.asarray(conv_b, dtype=np.float32)
    Wq = np.asarray(Wq, dtype=np.float32)
    Wv = np.asarray(Wv, dtype=np.float32)
    score_w = np.asarray(score_w, dtype=np.float32)
    score_b = np.asarray(score_b, dtype=np.float32)
    bias = np.asarray(bias, dtype=np.float32)

    # Shared (replicated) weight-side arrays.
    # wvt[p, kc, d] = Wv[d, kc*128+p]  (lhsT chunks for the projection)
    wvt = np.ascontiguousarray(Wv.T.reshape(KC, 128, H).transpose(1, 0, 2))
    wqt = np.ascontiguousarray(Wq.T.reshape(KC, 128, H).transpose(1, 0, 2))
    swt = np.ascontiguousarray(score_w[0].reshape(KC, 128).T)
    bc4 = np.ascontiguousarray((bias + conv_b).reshape(KC, 128).T)
    s3 = np.ascontiguousarray(conv_w[:, 0, :].T)  # s3[k, d] = conv_w[d, 0, k]
    sb = score_b.reshape(1, 1)

    # r[b, k, t] = la[b, t + k - 1] (zero-padded)
    r = np.zeros((B, 3, VL), dtype=np.float32)
    r[:, 0, 1:] = last_attn[:, :-1]
    r[:, 1, :] = last_attn
    r[:, 2, :-1] = last_attn[:, 1:]

    qT = query.reshape(B, H).T  # [h, b]

    in_maps = []
    for c in range(N_CORES):
        s = slice(c * BPC, (c + 1) * BPC)
        vshard = value[s]
        in_maps.append(
            {
                "valt": np.ascontiguousarray(vshard.transpose(0, 2, 1)),
                "vnat": vshard,
                "wvt": wvt,
                "wqt": wqt,
                "qt": np.ascontiguousarray(
                    qT[:, s].reshape(KC, 128, BPC).transpose(1, 0, 2)
                ),
                "swt": swt,
                "bc4": bc4,
                "s3": s3,
                "rmat": r[s],
                "sb": sb,
            }
        )
    return in_maps


def kernel(query, value, last_attn, conv_w, conv_b, Wq, Wv, score_w, score_b, bias,
           _trace=False):
    if "nc" not in _CACHE:
        _CACHE["nc"] = build_module()
    nc = _CACHE["nc"]

    in_maps = make_in_maps(
        query, value, last_attn, conv_w, conv_b, Wq, Wv, score_w, score_b, bias
    )
    res = run_bass_kernel_spmd(nc, in_maps, list(range(N_CORES)), trace=_trace)

    context = np.concatenate([res.results[c]["ctx"] for c in range(N_CORES)], axis=0)
    attn = np.concatenate([res.results[c]["attn"] for c in range(N_CORES)], axis=0)
    out = (context.reshape(B, 1, H).astype(np.float32), attn.astype(np.float32))
    if _trace:
        return out, res
    return out


# revision 9
# speedup vs baseline: 1.3334x; 1.0665x over previous
"""LocationAwareAttention Trainium2 kernel (8 NeuronCores, batch-data-parallel).

Reference computation (per batch b):
    conv[t, h] = sum_k conv_w[h, 0, k] * la[t + k - 1]          (padded conv1d)
    hidden[t, d] = tanh(q[d] + (value @ Wv^T)[t, d] + conv[t, d] + conv_b[d] + bias[d])
    score[t] = sigmoid(hidden[t, :] @ score_w + score_b)
    attn = score / sum(score);  context = attn @ value

Device dataflow (per core, 4 batches):
    hiddenT[d, t] = tanh( sum_h WvT[h, d] * valueT[h, t]        (PE, K=512 in 4 chunks)
                        + sum_k s3[k, d] * r[k, t]              (PE, K=3 conv fold)
                        + qb[d] )                               (ACT per-partition bias)
    score = M=1 PE matmuls over hiddenT; sigmoid+row-sum fused on ACT (accum_out)
    attn normalize on DVE; attn^T via a DRAM bounce; context = M=1 PE matmuls
    over naturally-laid-out value tiles.
value is shipped twice from host (transposed for the projection, natural for the
context) so no on-device transposes are needed.
"""

import os
import sys

try:
    import concourse  # noqa: F401
except ImportError:
    for _p in ("/opt/trn_rl_repo", "/root/.axon_site/_ro/trn_rl_repo"):
        if os.path.isdir(_p):
            sys.path.insert(0, _p)
            break

import numpy as np

import concourse.bacc as bacc
import concourse.mybir as mybir
import concourse.tile as tile
from concourse.bass_utils import run_bass_kernel_spmd

F32 = mybir.dt.float32
F32R = mybir.dt.float32r  # fp32 bytes; single-pass PE mode (4x faster than fp32 LOW_HIGH)
AF = mybir.ActivationFunctionType

N_CORES = 8
B, VL, H = 32, 2048, 512
BPC = B // N_CORES  # batches per core
KC = H // 128  # contraction chunks
TJ = VL // 512  # 512-wide t chunks

_CACHE = {}


def build_module():
    nc = bacc.Bacc("TRN2", target_bir_lowering=False, debug=False, num_devices=N_CORES)

    valt_d = nc.dram_tensor("valt", [BPC, H, VL], F32R, kind="ExternalInput")
    vnat_d = nc.dram_tensor("vnat", [BPC, VL, H], F32R, kind="ExternalInput")
    wvt_d = nc.dram_tensor("wvt", [128, KC, H], F32R, kind="ExternalInput")
    wqt_d = nc.dram_tensor("wqt", [128, KC, H], F32R, kind="ExternalInput")
    qt_d = nc.dram_tensor("qt", [128, KC, BPC], F32R, kind="ExternalInput")
    swt_d = nc.dram_tensor("swt", [128, KC], F32R, kind="ExternalInput")
    bc4_d = nc.dram_tensor("bc4", [128, KC], F32, kind="ExternalInput")
    s3_d = nc.dram_tensor("s3", [3, H], F32R, kind="ExternalInput")
    rmat_d = nc.dram_tensor("rmat", [BPC, 3, VL], F32R, kind="ExternalInput")
    sb_d = nc.dram_tensor("sb", [1, 1], F32, kind="ExternalInput")

    ctx_d = nc.dram_tensor("ctx", [BPC, H], F32, kind="ExternalOutput")
    attn_d = nc.dram_tensor("attn", [BPC, VL], F32, kind="ExternalOutput")

    vnr = vnat_d.ap().rearrange("b (tj j2 p) h -> b tj p j2 h", j2=4, p=128)

    with tile.TileContext(nc) as tc:
        with (
            tc.tile_pool(name="weights", bufs=1) as wpool,
            tc.tile_pool(name="vt", bufs=2 * KC) as vt_pool,
            tc.tile_pool(name="vn", bufs=2 * TJ) as vn_pool,
            tc.tile_pool(name="rb", bufs=2) as r_pool,
            tc.tile_pool(name="hid", bufs=3) as h_pool,
            tc.tile_pool(name="small", bufs=2) as sc_pool,
            tc.tile_pool(name="attnT", bufs=2) as at_pool,
            tc.tile_pool(name="dbounce", bufs=2, space="DRAM") as dram_pool,
            tc.tile_pool(name="pre_ps", bufs=2, space="PSUM") as ppool,
            tc.tile_pool(name="score_ps", bufs=2, space="PSUM") as spsum_pool,
            tc.tile_pool(name="ctx_ps", bufs=2, space="PSUM") as cpsum_pool,
            tc.tile_pool(name="q_ps", bufs=1, space="PSUM") as qpsum_pool,
        ):
            # --- load weights / small constants (once per core) ---
            wvt_sb = wpool.tile([128, KC, H], F32R)
            nc.sync.dma_start(wvt_sb[:], wvt_d[:])
            wqt_sb = wpool.tile([128, KC, H], F32R)
            nc.sync.dma_start(wqt_sb[:], wqt_d[:])
            qt_sb = wpool.tile([128, KC, BPC], F32R)
            nc.sync.dma_start(qt_sb[:], qt_d[:])
            swt_sb = wpool.tile([128, KC], F32R)
            nc.sync.dma_start(swt_sb[:], swt_d[:])
            bc4_sb = wpool.tile([128, KC], F32)
            nc.sync.dma_start(bc4_sb[:], bc4_d[:])
            s3_sb = wpool.tile([3, H], F32R)
            nc.sync.dma_start(s3_sb[:], s3_d[:])
            sb_sb = wpool.tile([1, 1], F32)
            nc.sync.dma_start(sb_sb[:], sb_d[:])

            # --- q projection for all batches: qT[d, b] = sum_h Wq[d,h] query[b,h]
            q_ps = qpsum_pool.tile([128, KC * BPC], F32)
            for dc in range(KC):
                for kc in range(KC):
                    nc.tensor.matmul(
                        q_ps[:, dc * BPC : (dc + 1) * BPC],
                        wqt_sb[:, kc, dc * 128 : (dc + 1) * 128],
                        qt_sb[:, kc, :],
                        start=(kc == 0),
                        stop=(kc == KC - 1),
                    )
            # qb[d, b] = qT[d, b] + bias[d] + conv_b[d]   (tanh bias, per-partition)
            qb_sb = wpool.tile([128, KC, BPC], F32)
            for dc in range(KC):
                nc.vector.tensor_tensor(
                    qb_sb[:, dc, :],
                    q_ps[:, dc * BPC : (dc + 1) * BPC],
                    bc4_sb[:, dc : dc + 1].to_broadcast((128, BPC)),
                    mybir.AluOpType.add,
                )

            def emit_loads(b, split_vt=False):
                # vt on Sync HWDGE, vn on GpSimd SWDGE: independent queues so a
                # blocked issue on one never head-of-line-blocks the other.
                vts, vns = [], []
                for hc in range(KC):
                    vt = vt_pool.tile([128, VL], F32R, tag="vt")
                    if split_vt:  # first batch: quarter-DMAs so PE starts sooner
                        for j in range(TJ):
                            nc.sync.dma_start(
                                vt[:, j * 512 : (j + 1) * 512],
                                valt_d[b, hc * 128 : (hc + 1) * 128,
                                       j * 512 : (j + 1) * 512],
                            )
                    else:
                        nc.sync.dma_start(
                            vt[:], valt_d[b, hc * 128 : (hc + 1) * 128, :]
                        )
                    vts.append(vt)
                for tj in range(TJ):
                    vn = vn_pool.tile([128, 4, 512], F32R, tag="vn")
                    nc.gpsimd.dma_start(vn[:], vnr[b, tj])
                    vns.append(vn)
                r_b = r_pool.tile([3, VL], F32R, tag="r_b")
                nc.sync.dma_start(r_b[:], rmat_d[b])
                return vts, vns, r_b

            loads = {0: emit_loads(0)}
            prev = None
            for b in range(BPC):
                vts, vns, r_b = loads.pop(b)
                if b + 1 < BPC:
                    # prefetch next batch now, so its DMAs sit AHEAD of this
                    # batch's bounce hops in both queues
                    loads[b + 1] = emit_loads(b + 1)

                score_sb = sc_pool.tile([1, VL], F32R, tag="score_sb")
                ssum4 = sc_pool.tile([1, TJ], F32, tag="ssum4")
                ab = dram_pool.tile([1, VL], F32R)
                attnT = at_pool.tile([128, VL // 128], F32R)
                sps = {}
                pending = None

                def flush_score(j, dc, hT):
                    nc.tensor.matmul(
                        sps[j][:],
                        swt_sb[:, dc : dc + 1],
                        hT[:],
                        start=(dc == 0),
                        stop=(dc == KC - 1),
                    )
                    if dc == KC - 1:
                        # sigmoid(score + score_b) with fused row-sum; then
                        # bounce raw scores toward attn^T layout. hop1 is on
                        # ACT right behind the sigmoid (never waits); hop2's
                        # 4B-gather runs on the idle GpSimd queue.
                        nc.scalar.activation(
                            score_sb[0:1, j * 512 : (j + 1) * 512],
                            sps[j][:],
                            AF.Sigmoid,
                            bias=sb_sb[0:1, :],
                            accum_out=ssum4[0:1, j : j + 1],
                        )
                        nc.scalar.dma_start(
                            ab[0:1, j * 512 : (j + 1) * 512],
                            score_sb[0:1, j * 512 : (j + 1) * 512],
                        )
                        nc.gpsimd.dma_start(
                            attnT[:, j * 4 : (j + 1) * 4],
                            ab[0:1, j * 512 : (j + 1) * 512].rearrange(
                                "1 (j2 p) -> p j2", p=128
                            ),
                        )

                for j in range(TJ):
                    sps[j] = spsum_pool.tile([1, 512], F32, tag="sp", name=f"sp_{b}_{j}")
                    for dc in range(KC):
                        pre = ppool.tile([128, 512], F32)
                        for kc in range(KC):
                            nc.tensor.matmul(
                                pre[:],
                                wvt_sb[:, kc, dc * 128 : (dc + 1) * 128],
                                vts[kc][:, j * 512 : (j + 1) * 512],
                                start=(kc == 0),
                                stop=False,
                            )
                        nc.tensor.matmul(
                            pre[:],
                            s3_sb[:, dc * 128 : (dc + 1) * 128],
                            r_b[:, j * 512 : (j + 1) * 512],
                            start=False,
                            stop=True,
                        )
                        hT = h_pool.tile([128, 512], F32R)
                        nc.scalar.activation(
                            hT[:], pre[:], AF.Tanh, bias=qb_sb[:, dc, b : b + 1]
                        )
                        # score matmul delayed one (j, dc) block so PE never
                        # waits on the tanh that feeds it
                        if pending is not None:
                            flush_score(*pending)
                        pending = (j, dc, hT)
                        # previous batch's context right after the first block,
                        # freeing its vn tiles for the next prefetch
                        if j == 1 and dc == KC - 1 and prev is not None:
                            _emit_ctx(nc, cpsum_pool, sc_pool, ctx_d, *prev)
                flush_score(*pending)

                # --- normalized attn output (off the ctx critical path) ---
                ssum = sc_pool.tile([1, 1], F32, tag="ssum")
                nc.vector.tensor_reduce(
                    ssum[:], ssum4[:], axis=mybir.AxisListType.X, op=mybir.AluOpType.add
                )
                recip = sc_pool.tile([1, 1], F32, tag="recip")
                nc.vector.reciprocal(recip[:], ssum[:])
                attn_sb = sc_pool.tile([1, VL], F32, tag="attn_sb")
                nc.vector.tensor_scalar_mul(attn_sb[:], score_sb[:], recip[0:1, :])
                nc.sync.dma_start(attn_d[b : b + 1, :], attn_sb[:])

                prev = (b, attnT, vns, recip)

            _emit_ctx(nc, cpsum_pool, sc_pool, ctx_d, *prev)

    nc.compile()
    return nc


def _emit_ctx(nc, cpsum_pool, sc_pool, ctx_d, b, attnT, vns, recip):
    # ctx_u = sum_t sigmoid_score[t] * value[t, :]; normalized by 1/sum at copy
    cp = cpsum_pool.tile([1, 512], F32)
    n = VL // 128
    for jj in range(n):
        nc.tensor.matmul(
            cp[:],
            attnT[:, jj : jj + 1],
            vns[jj // 4][:, jj % 4, :],
            start=(jj == 0),
            stop=(jj == n - 1),
        )
    ctx_sb = sc_pool.tile([1, 512], F32, tag="ctx_sb")
    nc.scalar.activation(ctx_sb[:], cp[:], AF.Copy, scale=recip[0:1, :])
    nc.sync.dma_start(ctx_d[b : b + 1, :], ctx_sb[:])


def make_in_maps(query, value, last_attn, conv_w, conv_b, Wq, Wv, score_w, score_b, bias):
    query = np.asarray(query, dtype=np.float32)
    value = np.ascontiguousarray(np.asarray(value, dtype=np.float32))
    last_attn = np.asarray(last_attn, dtype=np.float32)
    conv_w = np.asarray(conv_w, dtype=np.float32)
    conv_b = np.asarray(conv_b, dtype=np.float32)
    Wq = np.asarray(Wq, dtype=np.float32)
    Wv = np.asarray(Wv, dtype=np.float32)
    score_w = np.asarray(score_w, dtype=np.float32)
    score_b = np.asarray(score_b, dtype=np.float32)
    bias = np.asarray(bias, dtype=np.float32)

    # Shared (replicated) weight-side arrays.
    # wvt[p, kc, d] = Wv[d, kc*128+p]  (lhsT chunks for the projection)
    wvt = np.ascontiguousarray(Wv.T.reshape(KC, 128, H).transpose(1, 0, 2))
    wqt = np.ascontiguousarray(Wq.T.reshape(KC, 128, H).transpose(1, 0, 2))
    swt = np.ascontiguousarray(score_w[0].reshape(KC, 128).T)
    bc4 = np.ascontiguousarray((bias + conv_b).reshape(KC, 128).T)
    s3 = np.ascontiguousarray(conv_w[:, 0, :].T)  # s3[k, d] = conv_w[d, 0, k]
    sb = score_b.reshape(1, 1)

    # r[b, k, t] = la[b, t + k - 1] (zero-padded)
    r = np.zeros((B, 3, VL), dtype=np.float32)
    r[:, 0, 1:] = last_attn[:, :-1]
    r[:, 1, :] = last_attn
    r[:, 2, :-1] = last_attn[:, 1:]

    qT = query.reshape(B, H).T  # [h, b]

    in_maps = []
    for c in range(N_CORES):
        s = slice(c * BPC, (c + 1) * BPC)
        vshard = value[s]
        in_maps.append(
            {
                "valt": np.ascontiguousarray(vshard.transpose(0, 2, 1)),
                "vnat": vshard,
                "wvt": wvt,
                "wqt": wqt,
                "qt": np.ascontiguousarray(
                    qT[:, s].reshape(KC, 128, BPC).transpose(1, 0, 2)
                ),
                "swt": swt,
                "bc4": bc4,
                "s3": s3,
                "rmat": r[s],
                "sb": sb,
            }
        )
    return in_maps


def kernel(query, value, last_attn, conv_w, conv_b, Wq, Wv, score_w, score_b, bias,
           _trace=False):
    if "nc" not in _CACHE:
        _CACHE["nc"] = build_module()
    nc = _CACHE["nc"]

    in_maps = make_in_maps(
        query, value, last_attn, conv_w, conv_b, Wq, Wv, score_w, score_b, bias
    )
    res = run_bass_kernel_spmd(nc, in_maps, list(range(N_CORES)), trace=_trace)

    context = np.concatenate([res.results[c]["ctx"] for c in range(N_CORES)], axis=0)
    attn = np.concatenate([res.results[c]["attn"] for c in range(N_CORES)], axis=0)
    out = (context.reshape(B, 1, H).astype(np.float32), attn.astype(np.float32))
    if _trace:
        return out, res
    return out
